# revision 5
# baseline (speedup 1.0000x reference)
"""Trainium2 Bass kernel for nn_CLLoss_66537633349977.

Strategy (8 NeuronCores, data-parallel over rows of f_n):
  f_n = concat(f_l, f_u)  [4096, 1024]; each core owns 512 rows.
  Per core, on device (all matmuls in bf16 on the PE, fp32 PSUM accum):
    - l_neg logits block  = f_n_loc @ f_ab.T   [512, 8192]
      -> exp(10*x) on ScalarE with fused row-sum accumulation -> sneg [512]
    - Gram block          = f_n_loc @ f_n.T    [512, 4096]  (raw logits out)
    - self-dots           = rowsum(f_n_loc * f_n_noise_loc) in fp32 on VectorE
  Host: gathers the 16 positive logits per row from the Gram matrix
  (pos_idx is the fixed jax.random seed-42 permutation set, recomputed in
  pure numpy via an exact threefry2x32 clone), applies exp/log/mean in fp32
  with the same overflow semantics as the fp32 reference, and computes the
  tiny terms 0-2 directly.
"""

import numpy as np
import ml_dtypes

import concourse.bacc as bacc
import concourse.tile as tile
from concourse import mybir
from concourse import bass_utils

# ---- problem constants (hardcoded per spec) ----
N_CORES = 8
N_L, N_U, N_AB, N_ALL, D = 1024, 3072, 8192, 16384, 1024
N = N_L + N_U            # 4096 rows of f_n
N_LOC = N // N_CORES     # 512 rows per core
P = 128                  # SBUF partitions
FREE = 512               # matmul moving free dim / PSUM bank
KT = D // P              # 8 contraction tiles
MB = N_LOC // P          # 4 row blocks per core
NEG_J = N_AB // FREE     # 16 column chunks of l_neg
GRAM_J = N // FREE       # 8 column chunks of the Gram block
THRESHOLD = np.float32(0.1)
T_INV = 10.0             # 1/T
NUM_POS = 16

FP32 = mybir.dt.float32
BF16 = mybir.dt.bfloat16
BF = ml_dtypes.bfloat16


# ---------------------------------------------------------------------------
# pure-numpy clone of jax.random (threefry2x32, partitionable=True) for the
# fixed seed-42 positive-index set used by the reference
# ---------------------------------------------------------------------------
def _threefry2x32(k1, k2, x0, x1):
    """Exact threefry2x32/20 (Random123-KAT-verified); k1/k2/x0/x1 broadcastable."""
    k1 = np.asarray(k1, np.uint32)
    k2 = np.asarray(k2, np.uint32)
    ks = [k1, k2, (k1 ^ k2 ^ np.uint32(0x1BD11BDA)).astype(np.uint32)]
    x0 = (x0 + ks[0]).astype(np.uint32)
    x1 = (x1 + ks[1]).astype(np.uint32)
    rots = [(13, 15, 26, 6), (17, 29, 16, 24)]
    for i in range(5):
        for r in rots[i % 2]:
            x0 = (x0 + x1).astype(np.uint32)
            x1 = ((x1 << np.uint32(r)) | (x1 >> np.uint32(32 - r))).astype(np.uint32)
            x1 = (x0 ^ x1).astype(np.uint32)
        x0 = (x0 + ks[(i + 1) % 3]).astype(np.uint32)
        x1 = (x1 + ks[(i + 2) % 3] + np.uint32(i + 1)).astype(np.uint32)
    return x0, x1


def _tf_split(key, n):
    """jax.random.split under jax_threefry_partitionable=True (foldlike)."""
    b1, b2 = _threefry2x32(
        key[0], key[1], np.zeros(n, np.uint32), np.arange(n, dtype=np.uint32)
    )
    return np.stack([b1, b2], axis=1)


def _positive_indices_np(n, num_pos, seed=42):
    """Vectorized clone of reference._positive_indices (jax seed-42 perms)."""
    keys = _tf_split((np.uint32(0), np.uint32(seed)), n)
    k1, k2 = keys[:, 0], keys[:, 1]
    num_rounds = int(np.ceil(3 * np.log(max(1, n - 1)) / np.log(2**32 - 1)))
    x = np.broadcast_to(np.arange(n - 1), (n, n - 1)).copy()
    zero2 = np.zeros((1, 2), np.uint32)
    cnt2 = np.arange(2, dtype=np.uint32)[None, :]
    cntm = np.arange(n - 1, dtype=np.uint32)[None, :]
    zerom = np.zeros((1, n - 1), np.uint32)
    for _ in range(num_rounds):
        b1, b2 = _threefry2x32(k1[:, None], k2[:, None], zero2, cnt2)
        k1, k2 = b1[:, 0], b2[:, 0]
        sb1, sb2 = _threefry2x32(b1[:, 1:2], b2[:, 1:2], zerom, cntm)
        sort_keys = (sb1 ^ sb2).astype(np.uint32)
        order = np.argsort(sort_keys, axis=1, kind="stable")
        x = np.take_along_axis(x, order, axis=1)
    perms = x[:, :num_pos]
    rows = np.arange(n)[:, None]
    return perms + (perms >= rows)


_POS_IDX_CACHE = [None]


def _pos_idx():
    if _POS_IDX_CACHE[0] is None:
        _POS_IDX_CACHE[0] = _positive_indices_np(N, NUM_POS)
    return _POS_IDX_CACHE[0]


# ---------------------------------------------------------------------------
# device program (built once, cached)
# ---------------------------------------------------------------------------
_PROGRAM_CACHE = [None]


def _build_program():
    nc = bacc.Bacc(
        "TRN2",
        target_bir_lowering=False,
        debug=False,
        enable_asserts=True,
        num_devices=N_CORES,
    )
    lhsT_d = nc.dram_tensor("lhsT_own", [P, KT * FREE], BF16, kind="ExternalInput").ap()
    rhs_ab_d = nc.dram_tensor(
        "rhs_ab_t", [NEG_J, P, KT * FREE], BF16, kind="ExternalInput"
    ).ap()
    rhs_n_d = nc.dram_tensor(
        "rhs_n_t", [GRAM_J, P, KT * FREE], BF16, kind="ExternalInput"
    ).ap()
    fn_d = nc.dram_tensor("fn_loc", [MB, P, D], FP32, kind="ExternalInput").ap()
    fnn_d = nc.dram_tensor("fnn_loc", [MB, P, D], FP32, kind="ExternalInput").ap()

    g_d = nc.dram_tensor("g_out", [MB, P, N], FP32, kind="ExternalOutput").ap()
    sneg_d = nc.dram_tensor("sneg", [MB, P, 1], FP32, kind="ExternalOutput").ap()
    sself_d = nc.dram_tensor("sself", [MB, P, 1], FP32, kind="ExternalOutput").ap()

    EXPF = mybir.ActivationFunctionType.Exp

    with tile.TileContext(nc) as tc:
        with (
            tc.tile_pool(name="const", bufs=1) as constp,
            tc.tile_pool(name="rhs", bufs=4) as rhsp,
            tc.tile_pool(name="psum", bufs=8, space="PSUM") as psump,
            tc.tile_pool(name="evict", bufs=4) as evp,
            tc.tile_pool(name="sums", bufs=1) as sumsp,
            tc.tile_pool(name="sd", bufs=2) as sdp,
        ):
            lhsT = constp.tile([P, KT * FREE], BF16)
            nc.sync.dma_start(lhsT[:], lhsT_d[:])

            # accum_out partials: column m*NEG_J + j
            sneg_parts = sumsp.tile([P, MB * NEG_J], FP32)

            for j in range(NEG_J + GRAM_J):
                rhs = rhsp.tile([P, KT * FREE], BF16, tag="rhs")
                src = rhs_ab_d[j] if j < NEG_J else rhs_n_d[j - NEG_J]
                nc.sync.dma_start(rhs[:], src)
                for m in range(MB):
                    pt = psump.tile([P, FREE], FP32, tag="pt")
                    for kt in range(KT):
                        nc.tensor.matmul(
                            pt[:],
                            lhsT[:, kt * FREE + m * P : kt * FREE + (m + 1) * P],
                            rhs[:, kt * FREE : (kt + 1) * FREE],
                            start=(kt == 0),
                            stop=(kt == KT - 1),
                        )
                    if j < NEG_J:
                        ext = evp.tile([P, FREE], FP32, tag="exp")
                        col = m * NEG_J + j
                        nc.scalar.activation(
                            ext[:],
                            pt[:],
                            EXPF,
                            scale=T_INV,
                            accum_out=sneg_parts[:, col : col + 1],
                        )
                    else:
                        gt = evp.tile([P, FREE], FP32, tag="g")
                        nc.vector.tensor_copy(gt[:], pt[:])
                        jj = j - NEG_J
                        nc.sync.dma_start(
                            g_d[m, :, jj * FREE : (jj + 1) * FREE], gt[:]
                        )

            for m in range(MB):
                s = sdp.tile([P, 1], FP32, tag="sneg_red")
                nc.vector.reduce_sum(
                    s[:],
                    sneg_parts[:, m * NEG_J : (m + 1) * NEG_J],
                    axis=mybir.AxisListType.X,
                )
                nc.sync.dma_start(sneg_d[m], s[:])

            for m in range(MB):
                a = sdp.tile([P, D], FP32, tag="fn")
                nc.sync.dma_start(a[:], fn_d[m])
                b = sdp.tile([P, D], FP32, tag="fnn")
                nc.sync.dma_start(b[:], fnn_d[m])
                prod = sdp.tile([P, D], FP32, tag="prod")
                sd = sdp.tile([P, 1], FP32, tag="sdval")
                nc.vector.tensor_mul(prod[:], a[:], b[:])
                nc.vector.reduce_sum(sd[:], prod[:], axis=mybir.AxisListType.X)
                nc.sync.dma_start(sself_d[m], sd[:])

    nc.compile()
    return nc


def _program():
    if _PROGRAM_CACHE[0] is None:
        _PROGRAM_CACHE[0] = _build_program()
    return _PROGRAM_CACHE[0]


def _tile_rhs(x):
    """[R, D] fp32 -> bf16 tiled [R/FREE, P, KT*FREE]: [j, p, kt*FREE+n] = x[j*FREE+n, kt*P+p]."""
    r = x.shape[0]
    xb = x.astype(BF)
    xb = xb.reshape(r // FREE, FREE, KT, P)  # [j, n, kt, p]
    xb = np.ascontiguousarray(xb.transpose(0, 3, 2, 1))  # [j, p, kt, n]
    return xb.reshape(r // FREE, P, KT * FREE)


def kernel(**inputs):
    f_l = np.asarray(inputs["f_l"], dtype=np.float32)
    f_l_noise = np.asarray(inputs["f_l_noise"], dtype=np.float32)
    f_u = np.asarray(inputs["f_u"], dtype=np.float32)
    f_u_noise = np.asarray(inputs["f_u_noise"], dtype=np.float32)
    f_ab = np.asarray(inputs["f_ab"], dtype=np.float32)
    y = np.asarray(inputs["y"], dtype=np.float32)
    batch_y = np.asarray(inputs["batch_y"], dtype=np.float32)
    y_l = np.asarray(inputs["y_l"], dtype=np.float32)
    y_u = np.asarray(inputs["y_u"], dtype=np.float32)
    y_u_noise = np.asarray(inputs["y_u_noise"], dtype=np.float32)
    u_near = np.asarray(inputs["u_near"]).astype(np.int64)

    f_n = np.concatenate([f_l, f_u], axis=0)
    f_n_noise = np.concatenate([f_l_noise, f_u_noise], axis=0)

    rhs_ab_t = _tile_rhs(f_ab)      # [16, 128, 4096] bf16, replicated
    rhs_n_t = _tile_rhs(f_n)        # [8, 128, 4096] bf16, replicated

    nc = _program()
    in_maps = []
    for c in range(N_CORES):
        loc = slice(c * N_LOC, (c + 1) * N_LOC)
        in_maps.append(
            {
                "lhsT_own": rhs_n_t[c],  # N_LOC == FREE: core's own tiled block
                "rhs_ab_t": rhs_ab_t,
                "rhs_n_t": rhs_n_t,
                "fn_loc": f_n[loc].reshape(MB, P, D),
                "fnn_loc": f_n_noise[loc].reshape(MB, P, D),
            }
        )
    res = bass_utils.run_bass_kernel_spmd(
        nc, in_maps, core_ids=list(range(N_CORES))
    )

    g = np.concatenate(
        [res.results[c]["g_out"].reshape(N_LOC, N) for c in range(N_CORES)], axis=0
    )
    sneg = np.concatenate(
        [res.results[c]["sneg"].reshape(N_LOC) for c in range(N_CORES)]
    )
    sself = np.concatenate(
        [res.results[c]["sself"].reshape(N_LOC) for c in range(N_CORES)]
    )
    return _epilogue(
        batch_y, y_l, y_u, y_u_noise, y, u_near, g, sneg, sself
    )


def _epilogue(batch_y, y_l, y_u, y_u_noise, y, u_near, g, sneg, sself):
    # ---- host epilogue (fp32, same overflow semantics as the reference) ----
    with np.errstate(over="ignore", divide="ignore", invalid="ignore"):
        term0 = np.float32(np.sum((batch_y - y_l) ** 2) / np.float32(N_L))
        term1 = np.float32(np.sum((y_u - y_u_noise) ** 2) / np.float32(N_U))

        valid = u_near != -1
        gather = y[np.where(valid, u_near, 0)]
        diff = np.abs(y_u - gather)
        diff = np.where(diff < THRESHOLD, np.float32(0.0), diff)
        diff = np.where(valid, diff, np.float32(0.0))
        cnt = max(int(valid.sum()), 1)
        term2 = np.float32(np.sum(diff**2) / np.float32(cnt))

        pos_idx = _pos_idx()
        l_pos_rand = np.take_along_axis(g, pos_idx, axis=1).astype(np.float32)
        l_pos = np.exp(
            np.concatenate([l_pos_rand, sself[:, None]], axis=1) * np.float32(T_INV)
        ).astype(np.float32)
        pos_sum = l_pos.sum(axis=1)
        tot_sum = pos_sum + sneg
        term3 = np.float32(-np.mean(np.log(pos_sum / tot_sum)))

        total = np.float32(term0 + term1 + term2 + term3)

    return (
        np.float32(total),
        np.float32(term0),
        np.float32(term1),
        np.float32(term2),
        np.float32(term3),
    )


# revision 6
# speedup vs baseline: 732.3426x; 732.3426x over previous
"""Trainium2 Bass kernel for nn_CLLoss_66537633349977.

Strategy (8 NeuronCores, data-parallel over rows of f_n):
  f_n = concat(f_l, f_u)  [4096, 1024]; each core owns 512 rows.
  Per core, on device (all matmuls in bf16 on the PE, fp32 PSUM accum):
    - l_neg logits block  = f_n_loc @ f_ab.T   [512, 8192]
      -> exp(10*x) on ScalarE with fused row-sum accumulation -> sneg [512]
    - Gram block          = f_n_loc @ f_n.T    [512, 4096]  (raw logits out)
    - self-dots           = rowsum(f_n_loc * f_n_noise_loc) in fp32 on VectorE
  Host: gathers the 16 positive logits per row from the Gram matrix
  (pos_idx is the fixed jax.random seed-42 permutation set, recomputed in
  pure numpy via an exact threefry2x32 clone), applies exp/log/mean in fp32
  with the same overflow semantics as the fp32 reference, and computes the
  tiny terms 0-2 directly.
"""

import numpy as np
import ml_dtypes

import concourse.bacc as bacc
import concourse.tile as tile
from concourse import mybir
from concourse import bass_utils

# ---- problem constants (hardcoded per spec) ----
N_CORES = 8
N_L, N_U, N_AB, N_ALL, D = 1024, 3072, 8192, 16384, 1024
N = N_L + N_U            # 4096 rows of f_n
N_LOC = N // N_CORES     # 512 rows per core
P = 128                  # SBUF partitions
FREE = 512               # matmul moving free dim / PSUM bank
KT = D // P              # 8 contraction tiles
MB = N_LOC // P          # 4 row blocks per core
NEG_J = N_AB // FREE     # 16 column chunks of l_neg
GRAM_J = N // FREE       # 8 column chunks of the Gram block
THRESHOLD = np.float32(0.1)
T_INV = 10.0             # 1/T
NUM_POS = 16

FP32 = mybir.dt.float32
BF16 = mybir.dt.bfloat16
BF = ml_dtypes.bfloat16


# ---------------------------------------------------------------------------
# pure-numpy clone of jax.random (threefry2x32, partitionable=True) for the
# fixed seed-42 positive-index set used by the reference
# ---------------------------------------------------------------------------
def _threefry2x32(k1, k2, x0, x1):
    """Exact threefry2x32/20 (Random123-KAT-verified); k1/k2/x0/x1 broadcastable."""
    k1 = np.asarray(k1, np.uint32)
    k2 = np.asarray(k2, np.uint32)
    ks = [k1, k2, (k1 ^ k2 ^ np.uint32(0x1BD11BDA)).astype(np.uint32)]
    x0 = (x0 + ks[0]).astype(np.uint32)
    x1 = (x1 + ks[1]).astype(np.uint32)
    rots = [(13, 15, 26, 6), (17, 29, 16, 24)]
    for i in range(5):
        for r in rots[i % 2]:
            x0 = (x0 + x1).astype(np.uint32)
            x1 = ((x1 << np.uint32(r)) | (x1 >> np.uint32(32 - r))).astype(np.uint32)
            x1 = (x0 ^ x1).astype(np.uint32)
        x0 = (x0 + ks[(i + 1) % 3]).astype(np.uint32)
        x1 = (x1 + ks[(i + 2) % 3] + np.uint32(i + 1)).astype(np.uint32)
    return x0, x1


def _tf_split(key, n):
    """jax.random.split under jax_threefry_partitionable=True (foldlike)."""
    b1, b2 = _threefry2x32(
        key[0], key[1], np.zeros(n, np.uint32), np.arange(n, dtype=np.uint32)
    )
    return np.stack([b1, b2], axis=1)


def _positive_indices_np(n, num_pos, seed=42):
    """Vectorized clone of reference._positive_indices (jax seed-42 perms)."""
    keys = _tf_split((np.uint32(0), np.uint32(seed)), n)
    k1, k2 = keys[:, 0], keys[:, 1]
    num_rounds = int(np.ceil(3 * np.log(max(1, n - 1)) / np.log(2**32 - 1)))
    x = np.broadcast_to(np.arange(n - 1), (n, n - 1)).copy()
    zero2 = np.zeros((1, 2), np.uint32)
    cnt2 = np.arange(2, dtype=np.uint32)[None, :]
    cntm = np.arange(n - 1, dtype=np.uint32)[None, :]
    zerom = np.zeros((1, n - 1), np.uint32)
    for _ in range(num_rounds):
        b1, b2 = _threefry2x32(k1[:, None], k2[:, None], zero2, cnt2)
        k1, k2 = b1[:, 0], b2[:, 0]
        sb1, sb2 = _threefry2x32(b1[:, 1:2], b2[:, 1:2], zerom, cntm)
        sort_keys = (sb1 ^ sb2).astype(np.uint32)
        order = np.argsort(sort_keys, axis=1, kind="stable")
        x = np.take_along_axis(x, order, axis=1)
    perms = x[:, :num_pos]
    rows = np.arange(n)[:, None]
    return perms + (perms >= rows)


_POS_IDX_CACHE = [None]


def _pos_idx():
    if _POS_IDX_CACHE[0] is None:
        _POS_IDX_CACHE[0] = _positive_indices_np(N, NUM_POS)
    return _POS_IDX_CACHE[0]


# ---------------------------------------------------------------------------
# device program (built once, cached)
# ---------------------------------------------------------------------------
_PROGRAM_CACHE = [None]


def _build_program(reps=1):
    nc = bacc.Bacc(
        "TRN2",
        target_bir_lowering=False,
        debug=False,
        enable_asserts=True,
        num_devices=N_CORES,
    )
    lhsT_d = nc.dram_tensor("lhsT_own", [P, KT * FREE], BF16, kind="ExternalInput").ap()
    rhs_ab_d = nc.dram_tensor(
        "rhs_ab_t", [NEG_J, P, KT * FREE], BF16, kind="ExternalInput"
    ).ap()
    rhs_n_d = nc.dram_tensor(
        "rhs_n_t", [GRAM_J, P, KT * FREE], BF16, kind="ExternalInput"
    ).ap()
    fn_d = nc.dram_tensor("fn_loc", [MB, P, D], FP32, kind="ExternalInput").ap()
    fnn_d = nc.dram_tensor("fnn_loc", [MB, P, D], FP32, kind="ExternalInput").ap()

    g_d = nc.dram_tensor("g_out", [MB, P, N], FP32, kind="ExternalOutput").ap()
    sneg_d = nc.dram_tensor("sneg", [MB, P, 1], FP32, kind="ExternalOutput").ap()
    sself_d = nc.dram_tensor("sself", [MB, P, 1], FP32, kind="ExternalOutput").ap()

    EXPF = mybir.ActivationFunctionType.Exp

    rep_range = range(reps)
    with tile.TileContext(nc) as tc:
        with (
            tc.tile_pool(name="const", bufs=1) as constp,
            tc.tile_pool(name="rhs", bufs=4) as rhsp,
            tc.tile_pool(name="psum", bufs=8, space="PSUM") as psump,
            tc.tile_pool(name="evict", bufs=4) as evp,
            tc.tile_pool(name="sums", bufs=1) as sumsp,
            tc.tile_pool(name="sd", bufs=2) as sdp,
        ):
          for _rep in rep_range:
            lhsT = constp.tile([P, KT * FREE], BF16)
            nc.sync.dma_start(lhsT[:], lhsT_d[:])

            # accum_out partials: column m*NEG_J + j
            sneg_parts = sumsp.tile([P, MB * NEG_J], FP32)

            for j in range(NEG_J + GRAM_J):
                rhs = rhsp.tile([P, KT * FREE], BF16, tag="rhs")
                src = rhs_ab_d[j] if j < NEG_J else rhs_n_d[j - NEG_J]
                nc.sync.dma_start(rhs[:], src)
                for m in range(MB):
                    pt = psump.tile([P, FREE], FP32, tag="pt")
                    for kt in range(KT):
                        nc.tensor.matmul(
                            pt[:],
                            lhsT[:, kt * FREE + m * P : kt * FREE + (m + 1) * P],
                            rhs[:, kt * FREE : (kt + 1) * FREE],
                            start=(kt == 0),
                            stop=(kt == KT - 1),
                        )
                    if j < NEG_J:
                        ext = evp.tile([P, FREE], FP32, tag="exp")
                        col = m * NEG_J + j
                        nc.scalar.activation(
                            ext[:],
                            pt[:],
                            EXPF,
                            scale=T_INV,
                            accum_out=sneg_parts[:, col : col + 1],
                        )
                    else:
                        gt = evp.tile([P, FREE], FP32, tag="g")
                        nc.vector.tensor_copy(gt[:], pt[:])
                        jj = j - NEG_J
                        nc.sync.dma_start(
                            g_d[m, :, jj * FREE : (jj + 1) * FREE], gt[:]
                        )

            for m in range(MB):
                s = sdp.tile([P, 1], FP32, tag="sneg_red")
                nc.vector.reduce_sum(
                    s[:],
                    sneg_parts[:, m * NEG_J : (m + 1) * NEG_J],
                    axis=mybir.AxisListType.X,
                )
                nc.sync.dma_start(sneg_d[m], s[:])

            for m in range(MB):
                a = sdp.tile([P, D], FP32, tag="fn")
                nc.sync.dma_start(a[:], fn_d[m])
                b = sdp.tile([P, D], FP32, tag="fnn")
                nc.sync.dma_start(b[:], fnn_d[m])
                prod = sdp.tile([P, D], FP32, tag="prod")
                sd = sdp.tile([P, 1], FP32, tag="sdval")
                nc.vector.tensor_mul(prod[:], a[:], b[:])
                nc.vector.reduce_sum(sd[:], prod[:], axis=mybir.AxisListType.X)
                nc.sync.dma_start(sself_d[m], sd[:])

    nc.compile()
    return nc


def _program():
    if _PROGRAM_CACHE[0] is None:
        _PROGRAM_CACHE[0] = _build_program()
    return _PROGRAM_CACHE[0]


def _tile_rhs(x):
    """[R, D] fp32 -> bf16 tiled [R/FREE, P, KT*FREE]: [j, p, kt*FREE+n] = x[j*FREE+n, kt*P+p]."""
    r = x.shape[0]
    xb = x.astype(BF)
    xb = xb.reshape(r // FREE, FREE, KT, P)  # [j, n, kt, p]
    xb = np.ascontiguousarray(xb.transpose(0, 3, 2, 1))  # [j, p, kt, n]
    return xb.reshape(r // FREE, P, KT * FREE)


def kernel(**inputs):
    f_l = np.asarray(inputs["f_l"], dtype=np.float32)
    f_l_noise = np.asarray(inputs["f_l_noise"], dtype=np.float32)
    f_u = np.asarray(inputs["f_u"], dtype=np.float32)
    f_u_noise = np.asarray(inputs["f_u_noise"], dtype=np.float32)
    f_ab = np.asarray(inputs["f_ab"], dtype=np.float32)
    y = np.asarray(inputs["y"], dtype=np.float32)
    batch_y = np.asarray(inputs["batch_y"], dtype=np.float32)
    y_l = np.asarray(inputs["y_l"], dtype=np.float32)
    y_u = np.asarray(inputs["y_u"], dtype=np.float32)
    y_u_noise = np.asarray(inputs["y_u_noise"], dtype=np.float32)
    u_near = np.asarray(inputs["u_near"]).astype(np.int64)

    f_n = np.concatenate([f_l, f_u], axis=0)
    f_n_noise = np.concatenate([f_l_noise, f_u_noise], axis=0)

    rhs_ab_t = _tile_rhs(f_ab)      # [16, 128, 4096] bf16, replicated
    rhs_n_t = _tile_rhs(f_n)        # [8, 128, 4096] bf16, replicated

    nc = _program()
    in_maps = []
    for c in range(N_CORES):
        loc = slice(c * N_LOC, (c + 1) * N_LOC)
        in_maps.append(
            {
                "lhsT_own": rhs_n_t[c],  # N_LOC == FREE: core's own tiled block
                "rhs_ab_t": rhs_ab_t,
                "rhs_n_t": rhs_n_t,
                "fn_loc": f_n[loc].reshape(MB, P, D),
                "fnn_loc": f_n_noise[loc].reshape(MB, P, D),
            }
        )
    res = bass_utils.run_bass_kernel_spmd(
        nc, in_maps, core_ids=list(range(N_CORES))
    )

    g = np.concatenate(
        [res.results[c]["g_out"].reshape(N_LOC, N) for c in range(N_CORES)], axis=0
    )
    sneg = np.concatenate(
        [res.results[c]["sneg"].reshape(N_LOC) for c in range(N_CORES)]
    )
    sself = np.concatenate(
        [res.results[c]["sself"].reshape(N_LOC) for c in range(N_CORES)]
    )
    return _epilogue(
        batch_y, y_l, y_u, y_u_noise, y, u_near, g, sneg, sself
    )


def _epilogue(batch_y, y_l, y_u, y_u_noise, y, u_near, g, sneg, sself):
    # ---- host epilogue (fp32, same overflow semantics as the reference) ----
    with np.errstate(over="ignore", divide="ignore", invalid="ignore"):
        term0 = np.float32(np.sum((batch_y - y_l) ** 2) / np.float32(N_L))
        term1 = np.float32(np.sum((y_u - y_u_noise) ** 2) / np.float32(N_U))

        valid = u_near != -1
        gather = y[np.where(valid, u_near, 0)]
        diff = np.abs(y_u - gather)
        diff = np.where(diff < THRESHOLD, np.float32(0.0), diff)
        diff = np.where(valid, diff, np.float32(0.0))
        cnt = max(int(valid.sum()), 1)
        term2 = np.float32(np.sum(diff**2) / np.float32(cnt))

        pos_idx = _pos_idx()
        l_pos_rand = np.take_along_axis(g, pos_idx, axis=1).astype(np.float32)
        l_pos = np.exp(
            np.concatenate([l_pos_rand, sself[:, None]], axis=1) * np.float32(T_INV)
        ).astype(np.float32)
        pos_sum = l_pos.sum(axis=1)
        tot_sum = pos_sum + sneg
        term3 = np.float32(-np.mean(np.log(pos_sum / tot_sum)))

        total = np.float32(term0 + term1 + term2 + term3)

    return (
        np.float32(total),
        np.float32(term0),
        np.float32(term1),
        np.float32(term2),
        np.float32(term3),
    )
